# revision 28
# baseline (speedup 1.0000x reference)
"""Trainium2 Bass kernel for sliding-window GQA attention (VLM block).

Problem (hardcoded): B=2, T=S=2048, D=2048, N=16 q-heads, K=8 kv-heads,
H=128, G=2, rope base 10000, soft-cap 50, window 1024, causal prefill.

Sharding: 8 cores = 2 (batch) x 4 (head-groups). Core b*4+g handles batch b,
q-heads [4g,4g+4), kv-heads [2g,2g+2), and produces the partial output
x-projection for those heads; the host sums the 4 partials per batch
(the "output projection all-reduce" done host-side since I/O is full).

Device pipeline per core (per 512-token chunk c):
  A) QKV projections from pre-transposed x (contraction d on partitions),
     producing qT/kT [h, t] (wq stationary) and V [s, h] (x stationary).
     RoPE applied on eviction (rotation via SBUF->SBUF DMA across partitions).
  B) Flash attention, key-stationary: S^T[s, tau] = kT_j.T @ qT chunk,
     softcap tanh (ACT, PSUM->SBUF), band masks (DVE adds), exp (ACT),
     E^T zero-padded to full chunk width; PV accumulates enc^T[h, tau] over j
     in PSUM; denominator via ones-matmul (rows broadcast); normalize on
     PSUM->SBUF eviction with DVE reciprocal.
     No max-subtraction needed: logits are soft-capped to [-50, 50].
  C) Output projection: out[t, d] += enc^T slices (stationary) @ wo.

All matmuls run bf16 x bf16 -> fp32 PSUM (1 cycle/row on PE).
"""

import numpy as np
import ml_dtypes

import concourse.bass as bass
import concourse.mybir as mybir
import concourse.tile as tile
from concourse import bacc
from concourse.bass_utils import run_bass_kernel_spmd

F32 = mybir.dt.float32
BF16 = mybir.dt.bfloat16
MM_DT = BF16  # matmul operand dtype
NP_MM = ml_dtypes.bfloat16

B, T, D, H = 2, 2048, 2048, 128
NH, NKV = 16, 8           # total q heads / kv heads
HPC, KPC = 4, 2           # per-core q heads / kv heads
QUERY_SCALE = 0.08838834764831845
SOFT_CAP = 50.0
WINDOW = 1024
ROPE_BASE = 10000.0
TCH = 512                 # t-chunk
NCH = T // TCH            # 4 chunks
NTILE = T // 128          # 16 tiles
NEG = -100.0              # additive mask in tanh units; *50 => exp underflows to 0

AFT = mybir.ActivationFunctionType
DEBUG = False


def _build():
    nc = bacc.Bacc(None, target_bir_lowering=False)

    xT = nc.dram_tensor("xT", [D, T], MM_DT, kind="ExternalInput")
    wq = nc.dram_tensor("wq", [128, HPC, NTILE, 128], MM_DT, kind="ExternalInput")
    wk = nc.dram_tensor("wk", [128, KPC, NTILE, 128], MM_DT, kind="ExternalInput")
    wv = nc.dram_tensor("wv", [128, NTILE, KPC, 128], MM_DT, kind="ExternalInput")
    wo = nc.dram_tensor("wo", [128, HPC, D], MM_DT, kind="ExternalInput")
    cosf = nc.dram_tensor("cosf", [128, T], F32, kind="ExternalInput")
    sinf = nc.dram_tensor("sinf", [128, T], F32, kind="ExternalInput")
    mdiag = nc.dram_tensor("mdiag", [128, 128], MM_DT, kind="ExternalInput")
    mfar = nc.dram_tensor("mfar", [128, 128], MM_DT, kind="ExternalInput")
    ones = nc.dram_tensor("ones", [128, 128], MM_DT, kind="ExternalInput")
    out = nc.dram_tensor("out", [T, D], F32, kind="ExternalOutput")
    if DEBUG:
        dq = nc.dram_tensor("dq", [NCH, 128, HPC, TCH], BF16, kind="ExternalOutput")
        dk = nc.dram_tensor("dk", [NCH, 128, KPC, TCH], BF16, kind="ExternalOutput")
        dv = nc.dram_tensor("dv", [NCH, 128, 4, KPC, 128], BF16, kind="ExternalOutput")
        de = nc.dram_tensor("de", [NCH, 128, HPC, TCH], BF16, kind="ExternalOutput")

    with tile.TileContext(nc) as tc:
        with (
            tc.tile_pool(name="const", bufs=1) as cpool,
            tc.tile_pool(name="wts", bufs=1) as wpool,
            tc.tile_pool(name="proj", bufs=3) as ppool,
            tc.tile_pool(name="xin", bufs=30) as xpool,
            tc.tile_pool(name="kvs", bufs=5) as kvpool,
            tc.tile_pool(name="att", bufs=6) as apool,
            tc.tile_pool(name="tmp", bufs=4) as tpool,
            tc.tile_pool(name="psum", bufs=1, space="PSUM") as psum,
        ):
            # ---- constants / weights resident in SBUF
            cos_sb = cpool.tile([128, T], F32, tag="cos")
            sin_sb = cpool.tile([128, T], F32, tag="sin")
            md_sb = cpool.tile([128, 128], MM_DT, tag="md")
            mf_sb = cpool.tile([128, 128], MM_DT, tag="mf")
            on_sb = cpool.tile([128, 128], MM_DT, tag="on")
            nc.gpsimd.dma_start(cos_sb[:], cosf[:])
            nc.gpsimd.dma_start(sin_sb[:], sinf[:])
            nc.gpsimd.dma_start(md_sb[:], mdiag[:])
            nc.gpsimd.dma_start(mf_sb[:], mfar[:])
            nc.gpsimd.dma_start(on_sb[:], ones[:])

            wq01_sb = wpool.tile([128, 2, NTILE, 128], MM_DT, tag="wq01")
            wk_sb = wpool.tile([128, KPC, NTILE, 128], MM_DT, tag="wk")
            wq23_sb = wpool.tile([128, 2, NTILE, 128], MM_DT, tag="wq23")
            wv_sb = wpool.tile([128, NTILE, KPC, 128], MM_DT, tag="wv")
            wo_sb = wpool.tile([128, HPC, D], MM_DT, tag="wo")
            nc.scalar.dma_start(wq01_sb[:], wq[:, 0:2])
            nc.scalar.dma_start(wk_sb[:], wk[:])
            nc.scalar.dma_start(wq23_sb[:], wq[:, 2:4])
            nc.scalar.dma_start(wv_sb[:], wv[:])
            nc.gpsimd.dma_start(wo_sb[:], wo[:])

            def wq_slice(n, dt_):
                return (wq01_sb[:, n, dt_, :] if n < 2 else wq23_sb[:, n - 2, dt_, :])

            # per-chunk kT/V/qT kept for band history (bufs=4 covers c-2..c)
            kt_tiles = []   # [128, KPC, TCH] bf16, [h, kv, s]
            v_tiles = []    # [128, 4, KPC, 128] bf16, [s_r, stile, kv, h]
            enc_tiles = []

            def emit_wo(co, enc):
                # two d-chunks share each stationary enc slice: one weight
                # load feeds both PSUM banks (halves LDWEIGHTS on PE)
                for tt_ in range(4):
                    trow = 128 * (4 * co + tt_)
                    for dh in range(2):
                        o_a = psum.tile([128, TCH], F32, tag="b6", name="oa")
                        o_b = psum.tile([128, TCH], F32, tag="v", name="ob")
                        for n in range(HPC):
                            lhs = enc[:, n, 128 * tt_:128 * (tt_ + 1)]
                            st, sp = (n == 0), (n == HPC - 1)
                            nc.tensor.matmul(
                                o_a[:], lhs,
                                wo_sb[:, n, TCH * (2 * dh):TCH * (2 * dh + 1)],
                                start=st, stop=sp)
                            nc.tensor.matmul(
                                o_b[:], lhs,
                                wo_sb[:, n, TCH * (2 * dh + 1):TCH * (2 * dh + 2)],
                                start=st, stop=sp)
                        for half, ops in ((0, o_a), (1, o_b)):
                            dch = 2 * dh + half
                            og = tpool.tile([128, TCH], F32, tag="og", name="og")
                            nc.vector.tensor_copy(og[:], ops[:])
                            nc.sync.dma_start(
                                out[trow:trow + 128, TCH * dch:TCH * (dch + 1)],
                                og[:])

            for c in range(NCH):
                # ================= phase A: projections for chunk c =========
                xts = []
                for dt_ in range(NTILE):
                    xt = xpool.tile([128, TCH], MM_DT, tag="x")
                    nc.sync.dma_start(
                        xt[:], xT[128 * dt_:128 * (dt_ + 1), TCH * c:TCH * (c + 1)]
                    )
                    xts.append(xt)

                qt_c = ppool.tile([128, HPC, TCH], MM_DT, tag="qt")
                kt_c = kvpool.tile([128, KPC, TCH], MM_DT, tag="kt")
                cs = cos_sb[:, TCH * c:TCH * (c + 1)]
                sn = sin_sb[:, TCH * c:TCH * (c + 1)]

                def rope_evict(src, dst):
                    f = tpool.tile([128, TCH], F32, tag="ropef", name="f")
                    nc.vector.tensor_copy(f[:], src[:])
                    rot = tpool.tile([128, TCH], F32, tag="roper", name="rot")
                    nc.sync.dma_start(rot[0:64, :], f[64:128, :])
                    nc.sync.dma_start(rot[64:128, :], f[0:64, :])
                    a = tpool.tile([128, TCH], F32, tag="ropea", name="a")
                    nc.vector.tensor_mul(a[:], f[:], cs)
                    b_ = tpool.tile([128, TCH], F32, tag="ropeb", name="b_")
                    nc.vector.tensor_mul(b_[:], rot[:], sn)
                    nc.vector.tensor_add(dst, a[:], b_[:])

                # QK in two 3-bank sub-passes so phase A(c+1) can overlap B/C(c)
                groups = [((0, "q"), (1, "q"), (0, "k")), ((2, "q"), (3, "q"), (1, "k"))]
                banks = [("b0", "b1", "b4"), ("b2", "b3", "b5")]
                for gi, grp in enumerate(groups):
                    ps = [psum.tile([128, TCH], F32, tag=banks[gi][x], name=f"ps{x}")
                          for x in range(3)]
                    for dt_ in range(NTILE):
                        st, sp = (dt_ == 0), (dt_ == NTILE - 1)
                        for x, (idx, kind) in enumerate(grp):
                            w = wq_slice(idx, dt_) if kind == "q" else wk_sb[:, idx, dt_, :]
                            nc.tensor.matmul(ps[x][:], w, xts[dt_][:], start=st, stop=sp)
                    for x, (idx, kind) in enumerate(grp):
                        dst = qt_c[:, idx, :] if kind == "q" else kt_c[:, idx, :]
                        rope_evict(ps[x], dst)

                # V projection: one PSUM bank per s-subtile, serialized groups
                v_sb = kvpool.tile([128, 4, KPC, 128], MM_DT, tag="v_sb")
                for sl in range(4):
                    v_ps = psum.tile([128, KPC, 128], F32, tag="v", name=f"vps{sl}")
                    for dt_ in range(NTILE):
                        nc.tensor.matmul(
                            v_ps[:], xts[dt_][:, 128 * sl:128 * (sl + 1)],
                            wv_sb[:, dt_, :, :],
                            start=(dt_ == 0), stop=(dt_ == NTILE - 1))
                    nc.vector.tensor_copy(v_sb[:, sl, :, :], v_ps[:])
                v_tiles.append(v_sb)
                kt_tiles.append(kt_c)
                if DEBUG:
                    nc.sync.dma_start(dq[c], qt_c[:])
                    nc.sync.dma_start(dk[c], kt_c[:])
                    nc.sync.dma_start(dv[c], v_sb[:])

                # ================= phase B: attention for chunk c ============
                jmin, jmax = max(0, 4 * c - 8), 4 * c + 3
                enc_c = ppool.tile([128, HPC, TCH], MM_DT, tag="enc")
                for pair in range(2):
                    kv = pair
                    eb, db = (2, 4) if pair == 0 else (4, 2)
                    e_ps = [psum.tile([128, TCH], F32, tag=f"b{eb + i}", name=f"eps{i}") for i in range(2)]
                    d_ps = [psum.tile([128, TCH], F32, tag=f"b{db + i}", name=f"dps{i}") for i in range(2)]
                    for j in range(jmin, jmax + 1):
                        jr = j - 4 * c
                        w0, w1 = max(0, jr), min(3, jr + 8)
                        wd = (w1 - w0 + 1) * 128
                        cj, sl = j // 4, j % 4
                        st, sp = (j == jmin), (j == jmax)
                        for h2 in range(2):
                            n = 2 * pair + h2
                            sbank = ("b0", "b1", "b6")[(2 * (j - jmin) + h2) % 3]
                            s_ps = psum.tile([128, TCH], F32, tag=sbank, name="sps")
                            nc.tensor.matmul(
                                s_ps[:, :wd],
                                kt_tiles[cj][:, kv, 128 * sl:128 * (sl + 1)],
                                qt_c[:, n, 128 * w0:128 * w0 + wd],
                                start=True, stop=True)
                            tt = tpool.tile([128, TCH], F32, tag="tanh")
                            nc.scalar.activation(tt[:, :wd], s_ps[:, :wd], AFT.Tanh,
                                                 scale=QUERY_SCALE / SOFT_CAP)
                            e = apool.tile([128, TCH], MM_DT, tag=f"e{h2}")
                            nc.scalar.activation(e[:, 128 * w0:128 * w0 + wd],
                                                 tt[:, :wd], AFT.Exp, scale=SOFT_CAP)
                            if jr >= 0:  # diagonal causal mask (block w0)
                                bx = 128 * w0
                                nc.vector.tensor_mul(e[:, bx:bx + 128],
                                                     e[:, bx:bx + 128], md_sb[:])
                            if jr <= -5:  # far-edge window mask at block jr + 8
                                bx = 128 * (jr + 8)
                                nc.vector.tensor_mul(e[:, bx:bx + 128],
                                                     e[:, bx:bx + 128], mf_sb[:])
                            nc.tensor.matmul(
                                e_ps[h2][:, 128 * w0:128 * w0 + wd],
                                v_tiles[cj][:, sl, kv, :],
                                e[:, 128 * w0:128 * w0 + wd],
                                start=st, stop=sp)
                            nc.tensor.matmul(
                                d_ps[h2][:, 128 * w0:128 * w0 + wd], on_sb[:],
                                e[:, 128 * w0:128 * w0 + wd],
                                start=st, stop=sp)
                    for h2 in range(2):
                        n = 2 * pair + h2
                        rec = tpool.tile([128, TCH], F32, tag="rec")
                        nc.vector.reciprocal(rec[:], d_ps[h2][:])
                        nc.vector.tensor_mul(enc_c[:, n, :], e_ps[h2][:], rec[:])

                if DEBUG:
                    nc.sync.dma_start(de[c], enc_c[:])
                enc_tiles.append(enc_c)
                if c > 0:
                    emit_wo(c - 1, enc_tiles[c - 1])
            emit_wo(NCH - 1, enc_tiles[NCH - 1])
    nc.finalize()
    return nc


_CACHE = {}


def _host_inputs(x, wq, wkv, wo):
    """Build the 8 per-core input dicts (host-side reshape/transposes)."""
    pos = np.arange(T, dtype=np.float64)
    frac = 2.0 * np.arange(64, dtype=np.float64) / 128.0
    ts = ROPE_BASE ** frac
    ang = (pos[None, :] / ts[:, None]).astype(np.float32)  # [64, T]
    c64, s64 = np.cos(ang), np.sin(ang)
    cosf = np.concatenate([c64, c64], 0).astype(np.float32)
    sinf = np.concatenate([-s64, s64], 0).astype(np.float32)
    p = np.arange(128)
    mdiag = np.where(p[:, None] <= p[None, :], 1.0, 0.0).astype(NP_MM)
    mfar = np.where(p[:, None] > p[None, :], 1.0, 0.0).astype(NP_MM)
    ones = np.ones((128, 128), dtype=NP_MM)

    in_maps = []
    for core in range(8):
        b, g = divmod(core, 4)
        hs, ks = slice(4 * g, 4 * g + 4), slice(2 * g, 2 * g + 2)
        xTb = np.ascontiguousarray(x[b].T).astype(NP_MM)
        wq_r = np.ascontiguousarray(
            wq[hs].reshape(HPC, NTILE, 128, 128).transpose(2, 0, 1, 3)).astype(NP_MM)
        wk_r = np.ascontiguousarray(
            wkv[0, ks].reshape(KPC, NTILE, 128, 128).transpose(2, 0, 1, 3)).astype(NP_MM)
        wv_r = np.ascontiguousarray(
            wkv[1, ks].reshape(KPC, NTILE, 128, 128).transpose(2, 1, 0, 3)).astype(NP_MM)
        wo_r = np.ascontiguousarray(wo[hs].transpose(1, 0, 2)).astype(NP_MM)
        in_maps.append({
            "xT": xTb, "wq": wq_r, "wk": wk_r, "wv": wv_r, "wo": wo_r,
            "cosf": cosf, "sinf": sinf, "mdiag": mdiag, "mfar": mfar,
            "ones": ones,
        })
    return in_maps


def _run(x, wq, wkv, wo, trace=False):
    if "nc" not in _CACHE:
        _CACHE["nc"] = _build()
    nc = _CACHE["nc"]
    in_maps = _host_inputs(x, wq, wkv, wo)
    res = run_bass_kernel_spmd(nc, in_maps, core_ids=list(range(8)), trace=trace)
    outs = np.empty((B, T, D), dtype=np.float32)
    for b in range(B):
        outs[b] = sum(res.results[4 * b + g]["out"].astype(np.float64)
                      for g in range(4)).astype(np.float32)
    return outs, res


def kernel(x, segment_pos, attn_mask, wq, wkv, wo):
    outs, _ = _run(np.asarray(x), np.asarray(wq), np.asarray(wkv), np.asarray(wo))
    return outs
